# revision 20
# baseline (speedup 1.0000x reference)
"""Trainium2 Bass kernel for nn_EstimationGate: out = history_data * gate(node_emb).

Data-parallel over batch across 8 NeuronCores. Each core:
  1. computes the per-node gate MLP once (tiny: [2048,128]@[128,64] -> relu
     -> @[64,1] -> sigmoid),
  2. rearranges the gate into V[P, i] = gate[(P%16)*128 + i] (one tiled-
     identity matmul), matching the flat layout of 8 contiguous (b,t) slabs,
  3. streams its 48MB history shard through SBUF in 2MB contiguous chunks,
     multiplying on the vector engine against a zero-stride broadcast view
     of V (each gate value covers 32 channels).

DMA ring budget: each HWDGE ring sustains ~220GB/s, both together ~400GB/s
(HBM cap), so loads live on the sync ring and stores on the scalar ring,
with small setup traffic placed where it cannot delay either.
"""
import numpy as np

import concourse.bass as bass
import concourse.tile as tile
from concourse import bacc, masks, mybir
from concourse.bass_utils import run_bass_kernel_spmd

# Problem shape (hardcoded per spec).
N, E, H = 2048, 64, 64
B, T, C = 32, 48, 32
NCORES = 8
B_SH = B // NCORES            # 4 batches per core
SLAB = N * C                  # 65536 floats per (b,t) slab
KSLAB = 8                     # slabs per chunk -> 2MB chunks
FREE = 512 * KSLAB            # 4096 free dim
NCHUNK = (B_SH * T) // KSLAB  # 24 chunks per core
PS = 128 // KSLAB             # 16 partitions per slab inside a chunk
NODES_PER_PART = N // PS      # 128 nodes covered by one partition

F32 = mybir.dt.float32

_CACHE = {}


def _build_nc():
    nc = bacc.Bacc("TRN2", target_bir_lowering=False, debug=False)

    hist = nc.declare_dram_parameter("hist", [NCHUNK, 128, FREE], F32, isOutput=False)
    emb_u = nc.declare_dram_parameter("emb_u", [N, E], F32, isOutput=False)
    emb_d = nc.declare_dram_parameter("emb_d", [N, E], F32, isOutput=False)
    w1 = nc.declare_dram_parameter("w1", [2 * E, H], F32, isOutput=False)
    b1 = nc.declare_dram_parameter("b1", [H], F32, isOutput=False)
    w2 = nc.declare_dram_parameter("w2", [H, 1], F32, isOutput=False)
    b2 = nc.declare_dram_parameter("b2", [1], F32, isOutput=False)
    out = nc.declare_dram_parameter("out", [NCHUNK, 128, FREE], F32, isOutput=True)

    gate_dram = nc.dram_tensor("gate_scratch", [N], F32)

    with tile.TileContext(nc) as tc:
        with (
            tc.tile_pool(name="setup", bufs=1) as setup,
            tc.tile_pool(name="psum_tp", bufs=4, space="PSUM") as psum_tp,
            tc.tile_pool(name="psum2", bufs=2, space="PSUM") as psum2,
            tc.tile_pool(name="psum1", bufs=1, space="PSUM") as psum1,
            tc.tile_pool(name="main", bufs=8) as main,
        ):
            # ---- one-time gate computation -------------------------------
            # Natural contiguous embedding loads (scalar ring is idle at the
            # head; the sync ring fills with hist prefetches from t=0).
            nat_u = setup.tile([128, 16 * E], F32)
            nc.scalar.dma_start(nat_u[:], emb_u[:].rearrange("(p i) e -> p (i e)", p=128))
            nat_d = setup.tile([128, 16 * E], F32)
            nc.scalar.dma_start(nat_d[:], emb_d[:].rearrange("(p i) e -> p (i e)", p=128))

            identity = setup.tile([128, 128], F32)
            masks.make_identity(nc, identity[:])

            # featT[f, p*16+c] = feat[p*16+c, f]: 32 PE transposes of [128, E]
            # slices, written to strided node columns.
            featT = setup.tile([128, N], F32)
            ft_u = featT[0:E, :].rearrange("f (p c) -> f p c", c=16)
            ft_d = featT[E : 2 * E, :].rearrange("f (p c) -> f p c", c=16)
            for c in range(16):
                tp = psum_tp.tile([E, 128], F32, tag="tp")
                nc.tensor.transpose(tp[:], nat_u[:, c * E : (c + 1) * E], identity[:])
                nc.vector.tensor_copy(ft_u[:, :, c], tp[:])
            for c in range(16):
                tp = psum_tp.tile([E, 128], F32, tag="tp")
                nc.tensor.transpose(tp[:], nat_d[:, c * E : (c + 1) * E], identity[:])
                nc.vector.tensor_copy(ft_d[:, :, c], tp[:])

            w1_sb = setup.tile([2 * E, H], F32)
            nc.gpsimd.dma_start(w1_sb[:], w1[:])
            b1_sb = setup.tile([H, 1], F32)
            nc.gpsimd.dma_start(b1_sb[:], b1[:].rearrange("(p x) -> p x", x=1))
            w2_sb = setup.tile([H, 1], F32)
            nc.gpsimd.dma_start(w2_sb[:], w2[:])
            b2_sb = setup.tile([1, 1], F32)
            nc.gpsimd.dma_start(b2_sb[:], b2[:].rearrange("(p x) -> p x", x=1))

            # hiddenT[h, n] = relu(W1.T @ featT + b1)
            hiddenT = setup.tile([H, N], F32)
            for q in range(4):
                hp = psum2.tile([H, 512], F32, tag="hp")
                nc.tensor.matmul(
                    hp[:], w1_sb[:], featT[:, q * 512 : (q + 1) * 512],
                    start=True, stop=True,
                )
                nc.scalar.activation(
                    hiddenT[:, q * 512 : (q + 1) * 512], hp[:],
                    mybir.ActivationFunctionType.Relu, bias=b1_sb[:],
                )

            # gate[0, n] = sigmoid(W2.T @ hiddenT + b2)
            gate_sb = setup.tile([1, N], F32)
            for q in range(4):
                gp = psum1.tile([1, 512], F32, tag="gp")
                nc.tensor.matmul(
                    gp[:], w2_sb[:], hiddenT[:, q * 512 : (q + 1) * 512],
                    start=True, stop=True,
                )
                nc.scalar.activation(
                    gate_sb[:, q * 512 : (q + 1) * 512], gp[:],
                    mybir.ActivationFunctionType.Sigmoid, bias=b2_sb[:],
                )

            # bounce the gate row through DRAM to spread it over partitions
            nc.scalar.dma_start(gate_dram[:].rearrange("(x f) -> x f", x=1), gate_sb[:])
            gnat = setup.tile([PS, NODES_PER_PART], F32)
            nc.scalar.dma_start(gnat[:], gate_dram[:].rearrange("(q i) -> q i", q=PS))

            # V[P, i] = gnat[P % PS, i] via a tiled-identity matmul
            ti = setup.tile([PS, 128], F32)
            nc.vector.memset(ti[:], 1.0)
            nc.gpsimd.affine_select(
                out=ti[:].rearrange("m (r q) -> m r q", q=PS),
                in_=ti[:].rearrange("m (r q) -> m r q", q=PS),
                compare_op=mybir.AluOpType.is_equal, fill=0.0,
                base=0, pattern=[[0, 128 // PS], [1, PS]], channel_multiplier=-1,
            )
            vps = psum1.tile([128, NODES_PER_PART], F32, tag="vps")
            nc.tensor.matmul(vps[:], ti[:], gnat[:], start=True, stop=True)
            v_sb = setup.tile([128, NODES_PER_PART], F32)
            nc.vector.tensor_copy(v_sb[:], vps[:])
            v_bcast = v_sb[:].unsqueeze(-1).broadcast_to([128, NODES_PER_PART, C])

            # ---- streaming multiply -------------------------------------
            # First chunks: full-size load, but quarter-granular multiply +
            # store so the store ring saturates the moment V is ready.
            NHEAD = 4
            NTAIL = 2   # last chunks run in quarter pieces to shrink the tail
            QH = FREE // 4
            for i in range(NHEAD):
                t = main.tile([128, FREE], F32, tag="chunk")
                ld = nc.scalar if i < 2 else nc.sync
                ld.dma_start(t[:], hist[i])
                for s in range(4):
                    tv = t[:, s * QH : (s + 1) * QH].rearrange("p (i r) -> p i r", r=C)
                    nc.vector.tensor_mul(
                        tv, tv, v_bcast[:, s * (QH // C) : (s + 1) * (QH // C), :]
                    )
                    nc.scalar.dma_start(out[i][:, s * QH : (s + 1) * QH], t[:, s * QH : (s + 1) * QH])
            for i in range(NHEAD, NCHUNK - NTAIL):
                t = main.tile([128, FREE], F32, tag="chunk")
                st = nc.sync if i >= NCHUNK - NTAIL - 2 and i % 2 == 0 else nc.scalar
                nc.sync.dma_start(t[:], hist[i])
                tv = t[:].rearrange("p (i r) -> p i r", r=C)
                nc.vector.tensor_mul(tv, tv, v_bcast)
                st.dma_start(out[i], t[:])
            QF = FREE // 4
            for i in range(NCHUNK - NTAIL, NCHUNK):
                for s in range(4):
                    t = main.tile([128, QF], F32, tag="tail")
                    st = nc.sync if (i * 4 + s) % 2 == 0 else nc.scalar
                    nc.sync.dma_start(t[:], hist[i][:, s * QF : (s + 1) * QF])
                    tv = t[:].rearrange("p (i r) -> p i r", r=C)
                    nc.vector.tensor_mul(
                        tv, tv, v_bcast[:, s * (QF // C) : (s + 1) * (QF // C), :]
                    )
                    st.dma_start(out[i][:, s * QF : (s + 1) * QF], t[:])

    nc.compile()
    return nc


def _run(inputs, trace=False, trace_kwargs=None):
    if "nc" not in _CACHE:
        _CACHE["nc"] = _build_nc()
    nc = _CACHE["nc"]

    hist = np.ascontiguousarray(np.asarray(inputs["history_data"], dtype=np.float32))
    shards = hist.reshape(NCORES, NCHUNK, 128, FREE)
    common = {
        "emb_u": np.ascontiguousarray(np.asarray(inputs["node_embedding_u"], np.float32)),
        "emb_d": np.ascontiguousarray(np.asarray(inputs["node_embedding_d"], np.float32)),
        "w1": np.ascontiguousarray(np.asarray(inputs["W1"], np.float32)),
        "b1": np.ascontiguousarray(np.asarray(inputs["b1"], np.float32)),
        "w2": np.ascontiguousarray(np.asarray(inputs["W2"], np.float32)),
        "b2": np.ascontiguousarray(np.asarray(inputs["b2"], np.float32)),
    }
    in_maps = [{"hist": shards[i], **common} for i in range(NCORES)]
    kw = {}
    if trace:
        kw["trace"] = True
        if trace_kwargs:
            kw["trace_kwargs"] = trace_kwargs
    res = run_bass_kernel_spmd(nc, in_maps, list(range(NCORES)), **kw)
    out = np.concatenate(
        [r["out"].reshape(B_SH, T, N, C) for r in res.results], axis=0
    )
    return out, res


def kernel(**inputs):
    out, _ = _run(inputs)
    return out
